# revision 1
# baseline (speedup 1.0000x reference)
"""MultiHeadGAT layer as a Trainium2 Bass kernel (8-core SPMD), v3.

Design (N=50000, E=1.6M, F=256, HEADS=8, HD=32):
  - Host: permute nodes by in-degree; deal 128-node target tiles round-robin
    to 8 cores; per-tile slot cap (shared by all cores) + 1 sentinel slot.
    Each core gets its OWN node->table-row permutation (its targets first,
    in tile order) carried by its xT input + gather indices, so all DMA
    addresses are compile-time constants (SPMD-safe).
  - Phase A (every core): full node table computed locally (no AllGather).
    H = x@W + bw via PE from a transposed bf16 x; attention-logit linear
    terms si/sj (constants folded) computed in the same matmul; biases via a
    ones-row matmul + a DVE add.  Table row: 384 x bf16 (768 B) =
    [256 H | 16 sj-dup-pairs | 16 si-dup-pairs | 96 pad].  One extra pad row
    holds sj = -240 so padding slots vanish under exp() (no mask).
  - Phase B (per target tile): ONE dma_gather with *signed* int16 indices
    against a base-offset table view (base row 32768) covering all 50k rows
    (no lo/hi split); the gather's LAST index is the sentinel (non-negative)
    to dodge the trailing-negative DGE quirk.  Logits from gathered sj-dup;
    R = exp * H on DVE (bf16 2x packed); slot reduction via identity-weight
    PE matmuls in PSUM; normalize by 1/den post-reduction; skip + ELU +
    per-head LayerNorm + head-mean + output matmul (head-mean, gamma, beta,
    bout and the -mu*rstd LN correction folded into PE weights) + ELU;
    y stored bf16.
"""

import os
import sys

sys.path.insert(0, "/opt/trn_rl_repo")

import numpy as np
import ml_dtypes

import concourse.bass as bass
import concourse.bacc as bacc
import concourse.mybir as mybir
import concourse.tile as tile

# ---------------------------------------------------------------- constants
F_IN = 256
HID = 256
HEADS = 8
HD = 32
SLOPE = 0.2
EPS = 1e-5
P = 128
BASE = 32768

N_NODES = 50000
NC = 8
G_TILES = 392  # 392*128 = 50176 >= 50000; 392 % 8 == 0
NPAD = G_TILES * P
PADROW = NPAD
NR = NPAD + 16
TBW = 384  # bf16 elems per table row (768 B)
RB = 8
ABATCH = 4
NO_GATHER = False

F32 = mybir.dt.float32
BF16 = mybir.dt.bfloat16
F8 = mybir.dt.float8e4
I16 = mybir.dt.int16
AF = mybir.ActivationFunctionType
OP = mybir.AluOpType
AX = mybir.AxisListType

LT = G_TILES // NC


# ---------------------------------------------------------------- host prep
def _prepare(node_features, edge_index, W, bw, A, ba, gamma, beta, Wout, bout):
    x = np.asarray(node_features, np.float32)
    tgt = np.asarray(edge_index[0], np.int64)
    src = np.asarray(edge_index[1], np.int64)

    deg = np.bincount(tgt, minlength=N_NODES)
    perm = np.argsort(deg, kind="stable")
    rank = np.empty(N_NODES, np.int64)
    rank[perm] = np.arange(N_NODES)

    r_tgt = rank[tgt]
    r_src = rank[src]
    order = np.argsort(r_tgt, kind="stable")
    rt_s = r_tgt[order]
    rs_s = r_src[order]

    degp = np.bincount(rt_s, minlength=NPAD)
    starts = np.zeros(NPAD + 1, np.int64)
    np.cumsum(degp, out=starts[1:])
    dt_tile = degp.reshape(G_TILES, P).max(axis=1)
    # per-edge slot index within its (sorted) target group
    slot_of = np.arange(len(rt_s), dtype=np.int64) - starts[rt_s]

    # exact per-tile cap over the 8-core tile group (one shared Pool
    # register is rewritten before each gather, so caps need no rounding)
    tile_cap = np.zeros(LT, np.int64)
    sent = np.zeros(LT, np.int64)
    for t in range(LT):
        gs = [t * NC + c for c in range(NC)]
        tile_cap[t] = max(1, int(dt_tile[gs].max()))


    x_pad = np.zeros((NPAD, F_IN), np.float32)
    x_pad[:N_NODES] = x[perm]
    xTu = np.ascontiguousarray(
        x_pad.T.astype(ml_dtypes.bfloat16)
    ).view(np.uint16)

    # decide sentinel need per tile across all cores (order-invariant swap)
    for t in range(LT):
        cap = int(tile_cap[t])
        need = 0
        for c in range(NC):
            r = (t * NC + c) * P + 127
            dd = int(degp[r])
            if dd == cap:
                s0 = starts[r]
                if not (rs_s[s0 : s0 + dd] >= BASE).any():
                    need = 1
                    break
        sent[t] = need
    icols = int(8 * (tile_cap + sent).sum())

    ar = np.arange(NPAD)
    in_maps = []
    for c in range(NC):
        own = ((ar[: LT * P] // P) * NC + c) * P + (ar[: LT * P] % P)
        rest_mask = np.ones(NPAD, bool)
        rest_mask[own] = False
        pi_c = np.concatenate([own, ar[rest_mask]])
        rowc = np.empty(NPAD, np.int64)
        rowc[pi_c] = ar

        row_src = rowc[rs_s]
        idx_arr = np.zeros((P, icols), np.int16)
        icol = 0
        for t in range(LT):
            g = t * NC + c
            cap = int(tile_cap[t]) + int(sent[t])
            e0, e1 = starts[g * P], starts[(g + 1) * P]
            fl = np.full(P * cap, PADROW, np.int64)
            pp = rt_s[e0:e1] - g * P
            fl[slot_of[e0:e1] * P + pp] = row_src[e0:e1]
            if not sent[t]:
                # ensure the LAST index (slot cap-1, p=127) is non-negative:
                # if p127 fills all cap slots, swap one of its high-row
                # sources (row >= BASE; rowc permutes only rows < LT*P which
                # are all < BASE, so rank>=BASE <=> row>=BASE) into the end.
                last = (cap - 1) * P + 127
                if fl[last] < BASE:
                    own_slots = fl[127 :: P][:cap]
                    hi = np.where(own_slots >= BASE)[0]
                    assert len(hi), (t, c)
                    j = int(hi[0])
                    fl[last], fl[j * P + 127] = fl[j * P + 127], fl[last]
            i16 = (fl - BASE).astype(np.int16)
            idx_arr[:, icol : icol + 8 * cap] = np.tile(
                i16.reshape(-1, 16).T, (8, 1)
            )
            icol += 8 * cap

        xT = xTu[:, pi_c].view(ml_dtypes.bfloat16)
        in_maps.append(dict(idx=idx_arr, xT=xT.reshape(2, 128, NPAD)))

    # ---- weight-space folding (host, f64)
    W = np.asarray(W, np.float64)
    bw_ = np.asarray(bw, np.float64)
    A_ = np.asarray(A, np.float64)
    ba_ = np.asarray(ba, np.float64)
    gamma = np.asarray(gamma, np.float64)
    beta = np.asarray(beta, np.float64)
    Wout_ = np.asarray(Wout, np.float64)
    bout_ = np.asarray(bout, np.float64)

    Wcat = np.zeros((F_IN, HID))
    va1 = np.zeros((F_IN, HEADS))
    va2 = np.zeros((F_IN, HEADS))
    c1 = np.zeros(HEADS)
    c2 = np.zeros(HEADS)
    for h in range(HEADS):
        Wcat[:, h * HD : (h + 1) * HD] = W[h]
        va1[:, h] = W[h] @ A_[h, :HD]
        va2[:, h] = W[h] @ A_[h, HD:]
        c1[h] = bw_[h] @ A_[h, :HD] + ba_[h]
        c2[h] = bw_[h] @ A_[h, HD:]

    # ph columns: [0:256) H(no bias) | [256:272) sj dup | [272:288) si dup
    WCATA = np.zeros((2, 128, 288))
    BIASROW = np.zeros((1, 288))
    for k in range(2):
        WCATA[k, :, 0:256] = Wcat[k * 128 : (k + 1) * 128, :]
        for h in range(HEADS):
            for r in range(2):
                WCATA[k, :, 256 + 2 * h + r] = va2[k * 128 : (k + 1) * 128, h]
                WCATA[k, :, 272 + 2 * h + r] = va1[k * 128 : (k + 1) * 128, h]
    BIASROW[0, 128:256] = bw_.reshape(-1)[128:256]
    for h in range(HEADS):
        for r in range(2):
            BIASROW[0, 256 + 2 * h + r] = c2[h]
            BIASROW[0, 272 + 2 * h + r] = c1[h]

    WTILE = np.zeros((2, 128, HID))
    for f in range(F_IN):
        h, j = f // HD, f % HD
        WTILE[f // 128, f % 128, :] = gamma[h, j] * Wout_[j, :] / HEADS
    WEXTRA = np.zeros((9, HID))
    for h in range(HEADS):
        WEXTRA[h] = -(gamma[h] @ Wout_) / (HEADS * HD)
    WEXTRA[8] = bout_ + beta.mean(axis=0) @ Wout_

    padrow = np.zeros(TBW, ml_dtypes.bfloat16)
    padrow[256:272] = -240.0

    consts = dict(
        WCATA=WCATA.astype(ml_dtypes.bfloat16),
        BIASROW=BIASROW.astype(ml_dtypes.bfloat16),
        BWEXP=np.tile(bw_.reshape(1, -1), (P, 1)).astype(ml_dtypes.bfloat16),
        WTILE=WTILE.astype(ml_dtypes.bfloat16),
        WEXTRA=WEXTRA.astype(ml_dtypes.bfloat16),
        PADROW=padrow.reshape(1, TBW),
        ONESB=np.ones((1, 128), ml_dtypes.bfloat16),
        IDB=np.eye(P, dtype=ml_dtypes.bfloat16),
        IDF=np.eye(P, dtype=np.float32),
    )
    meta = dict(tile_cap=tile_cap, sent=sent, icols=icols, perm=perm)
    return meta, in_maps, consts


# ------------------------------------------------------------- device build
def _build(meta, consts):
    tile_cap, icols = meta["tile_cap"], meta["icols"]
    sent = meta["sent"]

    nc = bacc.Bacc(None, num_devices=NC)

    xT_d = nc.dram_tensor("xT", [2, 128, NPAD], BF16, kind="ExternalInput")
    idx_d = nc.dram_tensor("idx", [P, icols], I16, kind="ExternalInput")
    y_d = nc.dram_tensor("y", [LT * P, HID], BF16, kind="ExternalOutput")
    tbl_d = nc.dram_tensor("tbl", [NR, TBW], BF16)
    debug = os.environ.get("K2_DEBUG", "") == "1"
    if debug:
        dbg_tbl = nc.dram_tensor(
            "dbg_tbl", [NR, 2 * TBW], mybir.dt.uint8, kind="ExternalOutput"
        )

    cWCATA = nc.inline_tensor(np.asarray(consts["WCATA"]), "cWCATA")
    cBIASROW = nc.inline_tensor(np.asarray(consts["BIASROW"]), "cBIASROW")
    cBWEXP = nc.inline_tensor(np.asarray(consts["BWEXP"]), "cBWEXP")
    cWTILE = nc.inline_tensor(np.asarray(consts["WTILE"]), "cWTILE")
    cWEXTRA = nc.inline_tensor(np.asarray(consts["WEXTRA"]), "cWEXTRA")
    cPADROW = nc.inline_tensor(np.asarray(consts["PADROW"]), "cPADROW")
    cONESB = nc.inline_tensor(np.asarray(consts["ONESB"]), "cONESB")
    cIDB = nc.inline_tensor(np.asarray(consts["IDB"]), "cIDB")
    cIDF = nc.inline_tensor(np.asarray(consts["IDF"]), "cIDF")

    nidx_r = nc.alloc_register(mybir.EngineType.Pool, "nidx")

    with tile.TileContext(nc) as tc:
        with tc.tile_pool(name="const", bufs=1) as cpool:
            WCATA = cpool.tile([128, 2, 288], BF16)
            BIASROW = cpool.tile([1, 288], BF16)
            BWEXP = cpool.tile([P, 256], BF16)
            WTILE = cpool.tile([128, 2, HID], BF16)
            WEXTRA = cpool.tile([9, HID], BF16)
            PADT = cpool.tile([1, TBW], BF16)
            ONESB = cpool.tile([1, 128], BF16)
            IDB = cpool.tile([P, P], BF16)
            IDF = cpool.tile([P, P], F32)
            EPSC = cpool.tile([P, 1], F32)
            nc.gpsimd.memset(EPSC[:], EPS)
            nc.sync.dma_start(WCATA[:], cWCATA[:].rearrange("k p n -> p k n"))
            nc.sync.dma_start(BIASROW[:], cBIASROW[:])
            nc.sync.dma_start(BWEXP[:], cBWEXP[:])
            nc.sync.dma_start(WTILE[:], cWTILE[:].rearrange("k p n -> p k n"))
            nc.sync.dma_start(WEXTRA[:], cWEXTRA[:])
            nc.sync.dma_start(PADT[:], cPADROW[:])
            nc.sync.dma_start(ONESB[:], cONESB[:])
            nc.sync.dma_start(IDB[:], cIDB[:])
            nc.sync.dma_start(IDF[:], cIDF[:])

            # ================= Phase A: full node table ====================
            with (
                tc.tile_pool(name="xp", bufs=4) as xpool,
                tc.tile_pool(name="ap", bufs=4) as apool,
                tc.tile_pool(name="psA", bufs=2, space="PSUM") as psA,
            ):
                for b in range(G_TILES // ABATCH):
                    n0 = b * ABATCH * P
                    xb = xpool.tile([128, 2, ABATCH * P], BF16, tag="xb")
                    nc.sync.dma_start(
                        xb[:],
                        xT_d[:, :, n0 : n0 + ABATCH * P].rearrange(
                            "k p n -> p k n"
                        ),
                    )
                    ph = psA.tile([P, ABATCH, 512], F32, tag="phA")
                    for k in range(ABATCH):
                        nc.tensor.matmul(
                            ph[:, k, 0:288], xb[:, 0, k * P : (k + 1) * P],
                            WCATA[:, 0, :], start=True, stop=False,
                        )
                        nc.tensor.matmul(
                            ph[:, k, 0:128], xb[:, 1, k * P : (k + 1) * P],
                            WCATA[:, 1, 0:128], start=False, stop=True,
                        )
                        nc.tensor.matmul(
                            ph[:, k, 128:256], xb[:, 1, k * P : (k + 1) * P],
                            WCATA[:, 1, 128:256], start=False, stop=False,
                        )
                        nc.tensor.matmul(
                            ph[:, k, 256:288], xb[:, 1, k * P : (k + 1) * P],
                            WCATA[:, 1, 256:288], start=False, stop=False,
                        )
                        nc.tensor.matmul(
                            ph[:, k, 128:288], ONESB[:], BIASROW[:, 128:288],
                            start=False, stop=True,
                        )
                    t8 = apool.tile([P, ABATCH, 288], BF16, tag="t8")
                    nc.vector.tensor_tensor(
                        out=t8[:, :, 0:128],
                        in0=ph[:, :, 0:128],
                        in1=BWEXP[:, 0:128]
                        .unsqueeze(1)
                        .to_broadcast([P, ABATCH, 128]),
                        op=OP.add,
                    )
                    nc.scalar.copy(t8[:, :, 128:288], ph[:, :, 128:288])
                    nc.sync.dma_start(
                        tbl_d[n0 : n0 + ABATCH * P, 0:288].rearrange(
                            "(k p) w -> p k w", p=P
                        ),
                        t8[:],
                    )
                nc.sync.dma_start(tbl_d[PADROW : PADROW + 1, :], PADT[:])
                if debug:
                    nc.sync.dma_start(
                        dbg_tbl[:], tbl_d[:].bitcast(mybir.dt.uint8)
                    )

            # ================= Phase B =====================================
            with (
                tc.tile_pool(name="sp", bufs=3) as spool,
                tc.tile_pool(name="gp", bufs=2) as gpool,
                tc.tile_pool(name="rp", bufs=2) as rpool,
                tc.tile_pool(name="pp", bufs=2) as ppool,
                tc.tile_pool(name="psB", bufs=2, space="PSUM") as psB,
                tc.tile_pool(name="psC", bufs=2, space="PSUM") as psC,
            ):
                n_pairs = (LT + 1) // 2
                icol = 0
                for pi in range(n_pairs):
                    tiles = [t for t in (2 * pi, 2 * pi + 1) if t < LT]
                    ntl = len(tiles)

                    hblk = spool.tile([P, 2, 288], BF16, tag="hblk")
                    den = spool.tile([P, 2, HEADS], F32, tag="den")
                    pagg = psB.tile([P, 2, HID], F32, tag="ps_big")
                    for ti, t in enumerate(tiles):
                        cap = int(tile_cap[t]) + int(sent[t])
                        rcap = int(tile_cap[t])
                        nblk = (rcap + RB - 1) // RB
                        nc.sync.dma_start(
                            hblk[:, ti, :], tbl_d[t * P : (t + 1) * P, 0:288]
                        )
                        idxt = spool.tile([P, 8 * cap], I16, tag="idxt")
                        nc.sync.dma_start(
                            idxt[:], idx_d[:, icol : icol + 8 * cap]
                        )
                        icol += 8 * cap

                        grid = gpool.tile([P, cap, TBW], BF16, tag="grid")
                        if NO_GATHER:
                            nc.gpsimd.memset(grid[:], 0)
                        else:
                            nc.gpsimd.reg_mov(nidx_r, P * cap)
                            nc.gpsimd.dma_gather(
                                grid[:],
                                tbl_d[BASE:, :],
                                idxt[:],
                                P * cap,
                                nidx_r,
                                TBW,
                                single_packet=False,
                            )

                        egd = spool.tile([P, cap, 16], BF16, tag="egd")
                        nc.vector.tensor_tensor(
                            out=egd[:],
                            in0=grid[:, :, 256:272],
                            in1=hblk[:, ti, 272:288]
                            .unsqueeze(1)
                            .to_broadcast([P, cap, 16]),
                            op=OP.add,
                        )
                        exd = spool.tile([P, cap, 16], BF16, tag="exd")
                        nc.scalar.activation(
                            exd[:], egd[:], AF.Prelu, alpha=SLOPE
                        )
                        nc.scalar.activation(exd[:], exd[:], AF.Exp)
                        nc.vector.tensor_reduce(
                            den[:, ti, :],
                            exd[:]
                            .rearrange("p c (h two) -> p h two c", two=2)[
                                :, :, 0, :
                            ],
                            axis=AX.X,
                            op=OP.add,
                        )

                        # R = exp * H ; PE reduces slots (sentinel excluded)
                        ci = 0
                        for bb in range(nblk):
                            j0 = bb * RB
                            nb = min(RB, rcap - j0)
                            Rc = rpool.tile([P, RB, HID], BF16, tag="R")
                            nc.vector.tensor_tensor(
                                out=Rc[:, 0:nb, :].rearrange(
                                    "p c (h f two) -> p c h f two",
                                    h=HEADS, two=2,
                                ),
                                in0=grid[:, j0 : j0 + nb, 0:256].rearrange(
                                    "p c (h f two) -> p c h f two",
                                    h=HEADS, two=2,
                                ),
                                in1=exd[:, j0 : j0 + nb, :]
                                .rearrange("p c (h two) -> p c h two", two=2)
                                .unsqueeze(3)
                                .to_broadcast([P, nb, HEADS, HD // 2, 2]),
                                op=OP.mult,
                            )
                            for j in range(nb):
                                nc.tensor.matmul(
                                    pagg[:, ti, :],
                                    IDB[:],
                                    Rc[:, j, :],
                                    start=(ci == 0),
                                    stop=(ci == rcap - 1),
                                )
                                ci += 1

                    nc.vector.tensor_scalar_max(den[:], den[:], 1e-30)
                    rden = spool.tile([P, 2, HEADS], F32, tag="rden")
                    nc.vector.reciprocal(rden[:], den[:])

                    # ---- post (per pair): normalize, skip, ELU, LN, out, ELU
                    ob = ppool.tile([P, 2, HID], BF16, tag="ob")
                    nc.vector.tensor_tensor(
                        out=ob[:, 0:ntl, :].rearrange(
                            "p t (h f) -> p t h f", h=HEADS
                        ),
                        in0=pagg[:, 0:ntl, :].rearrange(
                            "p t (h f) -> p t h f", h=HEADS
                        ),
                        in1=rden[:, 0:ntl, :]
                        .unsqueeze(3)
                        .to_broadcast([P, ntl, HEADS, HD]),
                        op=OP.mult,
                    )
                    nc.vector.tensor_tensor(
                        out=ob[:, 0:ntl, :],
                        in0=ob[:, 0:ntl, :],
                        in1=hblk[:, 0:ntl, 0:256],
                        op=OP.add,
                    )
                    t1 = ppool.tile([P, 2, HID], BF16, tag="t1")
                    nc.scalar.activation(
                        t1[:, 0:ntl, :], ob[:, 0:ntl, :], AF.Relu, scale=-1.0
                    )
                    nc.scalar.activation(
                        t1[:, 0:ntl, :], t1[:, 0:ntl, :], AF.Exp, scale=-1.0
                    )
                    elu = ppool.tile([P, 2, HID], BF16, tag="elu")
                    nc.vector.scalar_tensor_tensor(
                        out=elu[:, 0:ntl, :],
                        in0=t1[:, 0:ntl, :],
                        scalar=-1.0,
                        in1=ob[:, 0:ntl, :],
                        op0=OP.add,
                        op1=OP.max,
                    )

                    nh = ntl * HEADS
                    st = ppool.tile([P, 8, 2 * HEADS], F32, tag="st")
                    r1, r2, mu2, var, sd, rr, tmp, _ = (
                        st[:, i, :] for i in range(8)
                    )
                    nc.vector.tensor_reduce(
                        r1[:, 0:nh],
                        elu[:, 0:ntl, :].rearrange(
                            "p t (h f) -> p (t h) f", f=HD
                        ),
                        axis=AX.X,
                        op=OP.add,
                    )
                    sq = ppool.tile([P, 2, HID], BF16, tag="t1")
                    nc.scalar.activation(
                        sq[:, 0:ntl, :], elu[:, 0:ntl, :], AF.Square
                    )
                    nc.vector.tensor_reduce(
                        r2[:, 0:nh],
                        sq[:, 0:ntl, :].rearrange(
                            "p t (h f) -> p (t h) f", f=HD
                        ),
                        axis=AX.X,
                        op=OP.add,
                    )
                    nc.scalar.activation(
                        mu2[:, 0:nh], r1[:, 0:nh], AF.Square, scale=1.0 / HD
                    )
                    nc.vector.scalar_tensor_tensor(
                        out=var[:, 0:nh], in0=r2[:, 0:nh], scalar=1.0 / HD,
                        in1=mu2[:, 0:nh], op0=OP.mult, op1=OP.subtract,
                    )
                    nc.scalar.activation(
                        sd[:, 0:nh], var[:, 0:nh], AF.Sqrt, bias=EPSC[:]
                    )
                    nc.vector.reciprocal(rr[:, 0:nh], sd[:, 0:nh])
                    nc.vector.tensor_tensor(
                        out=tmp[:, 0:nh], in0=r1[:, 0:nh], in1=rr[:, 0:nh],
                        op=OP.mult,
                    )

                    xw = ppool.tile([P, 2, HID], BF16, tag="xw")
                    nc.vector.tensor_tensor(
                        out=xw[:, 0:ntl, :].rearrange(
                            "p t (h f) -> p t h f", h=HEADS
                        ),
                        in0=elu[:, 0:ntl, :].rearrange(
                            "p t (h f) -> p t h f", h=HEADS
                        ),
                        in1=rr[:, 0:nh]
                        .rearrange("p (t h) -> p t h", h=HEADS)
                        .unsqueeze(3)
                        .to_broadcast([P, ntl, HEADS, HD]),
                        op=OP.mult,
                    )

                    py = psB.tile([P, 2, HID], F32, tag="ps_big")
                    yb = ppool.tile([P, 2, HID], BF16, tag="yb")
                    for ti, t in enumerate(tiles):
                        xwT = spool.tile([P, 2, P], BF16, tag="xwT")
                        for k in range(2):
                            pt = psC.tile([P, P], BF16, tag="ps_tr")
                            nc.tensor.transpose(
                                pt[:], xw[:, ti, k * P : (k + 1) * P], IDB[:]
                            )
                            nc.scalar.copy(xwT[:, k, :], pt[:])
                        t9 = spool.tile([P, 9], F32, tag="t9")
                        nc.scalar.copy(
                            t9[:, 0:8], tmp[:, ti * HEADS : (ti + 1) * HEADS]
                        )
                        nc.scalar.activation(
                            t9[:, 8:9], t9[:, 0:1], AF.Copy,
                            scale=0.0, bias=1.0,
                        )
                        ptm = psC.tile([P, P], F32, tag="ps_trf")
                        nc.tensor.transpose(ptm[0:9, :], t9[:], IDF[:])
                        exT = spool.tile([9, P], BF16, tag="exT")
                        nc.scalar.copy(exT[:], ptm[0:9, :])

                        nc.tensor.matmul(
                            py[:, ti, :], xwT[:, 0, :], WTILE[:, 0, :],
                            start=True, stop=False,
                        )
                        nc.tensor.matmul(
                            py[:, ti, :], xwT[:, 1, :], WTILE[:, 1, :],
                            start=False, stop=False,
                        )
                        nc.tensor.matmul(
                            py[:, ti, :], exT[:], WEXTRA[:],
                            start=False, stop=True,
                        )
                    nc.scalar.copy(yb[:, 0:ntl, :], py[:, 0:ntl, :])
                    t2 = ppool.tile([P, 2, HID], BF16, tag="t2")
                    nc.scalar.activation(
                        t2[:, 0:ntl, :], yb[:, 0:ntl, :], AF.Relu, scale=-1.0
                    )
                    nc.scalar.activation(
                        t2[:, 0:ntl, :], t2[:, 0:ntl, :], AF.Exp, scale=-1.0
                    )
                    ysb = ppool.tile([P, 2, HID], BF16, tag="ysb")
                    nc.vector.scalar_tensor_tensor(
                        out=ysb[:, 0:ntl, :],
                        in0=t2[:, 0:ntl, :],
                        scalar=-1.0,
                        in1=yb[:, 0:ntl, :],
                        op0=OP.add,
                        op1=OP.max,
                    )
                    for ti, t in enumerate(tiles):
                        nc.sync.dma_start(
                            y_d[t * P : (t + 1) * P, :], ysb[:, ti, :]
                        )

    nc.compile()
    return nc


# ------------------------------------------------------------------ driver
_CACHE = {}


def kernel(**inputs):
    meta, in_maps, consts = _prepare(**inputs)
    key = (
        tuple(meta["tile_cap"].tolist()),
        tuple(meta["sent"].tolist()),
    )
    if key not in _CACHE:
        _CACHE[key] = _build(meta, consts)
    nc = _CACHE[key]

    from concourse.bass_utils import run_bass_kernel_spmd

    global LAST_NC, LAST_INMAPS
    LAST_NC = nc
    LAST_INMAPS = in_maps

    res = run_bass_kernel_spmd(nc, in_maps, core_ids=list(range(NC)))
    global LAST_RESULT
    LAST_RESULT = res
    outs = res.results

    y_all = np.zeros((NPAD, HID), np.float32)
    for c in range(NC):
        g_idx = (np.arange(LT) * NC + c)[:, None] * P + np.arange(P)[None, :]
        y_all[g_idx.reshape(-1)] = outs[c]["y"].astype(np.float32)
    y = np.zeros((N_NODES, HID), np.float32)
    y[meta["perm"]] = y_all[:N_NODES]
    return y



# revision 3
# speedup vs baseline: 12.3371x; 12.3371x over previous
"""MultiHeadGAT Trainium2 Bass kernel (8-core SPMD), v4.

Changes vs v3 baseline: the dominant cost in the harness measurement is
per-exec input staging (v3 re-uploads the full 25.7MB bf16 node table to
every core, 205MB aggregate).  v4 uploads a 1/8 node-shard per core
(3.2MB) and reconstructs the full transposed feature table on device with
an AllGather collective; the gather index stream is uploaded un-replicated
([16, icols]) and replicated to the 32 partitions the SWDGE q0 reader
needs with two tiny DMAs, then kept SBUF-resident (no per-tile idx DMAs).

The node table lives in one canonical (degree-sorted) order shared by all
cores.  Since per-core target rows are no longer "own rows first", the
per-tile target block (hblk) is fetched with a small dma_gather whose
indices (per-core data) carry the 16-wrapped own-row list; 16 trailing
sentinel indices >= BASE dodge the trailing-negative DGE trim.

Everything else (weight-space folding, slot-capped edge gather, PE slot
reduction, fused LN/out-projection) is unchanged from v3.
"""

import os
import sys

sys.path.insert(0, "/opt/trn_rl_repo")

import numpy as np
import ml_dtypes

import concourse.bass as bass
import concourse.bacc as bacc
import concourse.mybir as mybir
import concourse.tile as tile

# ---------------------------------------------------------------- constants
F_IN = 256
HID = 256
HEADS = 8
HD = 32
SLOPE = 0.2
EPS = 1e-5
P = 128
BASE = 32768

N_NODES = 50000
NC = 8
G_TILES = 392  # 392*128 = 50176 >= 50000; 392 % 8 == 0
NPAD = G_TILES * P
PADROW = NPAD
NR = NPAD + 16
TBW = 384  # bf16 elems per table row (768 B)
RB = 8
SH = NPAD // NC  # 6272 nodes per shard

F32 = mybir.dt.float32
BF16 = mybir.dt.bfloat16
I16 = mybir.dt.int16
AF = mybir.ActivationFunctionType
OP = mybir.AluOpType
AX = mybir.AxisListType

LT = G_TILES // NC


# ---------------------------------------------------------------- host prep
def _prepare(node_features, edge_index, W, bw, A, ba, gamma, beta, Wout, bout):
    x = np.asarray(node_features, np.float32)
    tgt = np.asarray(edge_index[0], np.int64)
    src = np.asarray(edge_index[1], np.int64)

    deg = np.bincount(tgt, minlength=N_NODES)
    perm = np.argsort(deg, kind="stable")
    rank = np.empty(N_NODES, np.int64)
    rank[perm] = np.arange(N_NODES)

    r_tgt = rank[tgt]
    r_src = rank[src]
    order = np.argsort(r_tgt, kind="stable")
    rt_s = r_tgt[order]
    rs_s = r_src[order]

    degp = np.bincount(rt_s, minlength=NPAD)
    starts = np.zeros(NPAD + 1, np.int64)
    np.cumsum(degp, out=starts[1:])
    dt_tile = degp.reshape(G_TILES, P).max(axis=1)
    slot_of = np.arange(len(rt_s), dtype=np.int64) - starts[rt_s]

    tile_cap = np.zeros(LT, np.int64)
    sent = np.zeros(LT, np.int64)
    for t in range(LT):
        gs = [t * NC + c for c in range(NC)]
        tile_cap[t] = max(1, int(dt_tile[gs].max()))

    x_pad = np.zeros((NPAD, F_IN), np.float32)
    x_pad[:N_NODES] = x[perm]
    xT = np.ascontiguousarray(x_pad.T.astype(ml_dtypes.bfloat16))

    # sentinel need per tile (canonical rows: rank >= BASE <=> row >= BASE)
    for t in range(LT):
        cap = int(tile_cap[t])
        need = 0
        for c in range(NC):
            r = (t * NC + c) * P + 127
            dd = int(degp[r])
            if dd == cap:
                s0 = starts[r]
                if not (rs_s[s0 : s0 + dd] >= BASE).any():
                    need = 1
                    break
        sent[t] = need
    # columns: per tile 9 (own-gather 144 idx) + 8*(cap+sent) (edge gather)
    icols = int(9 * LT + 8 * (tile_cap + sent).sum())

    in_maps = []
    for c in range(NC):
        idx_arr = np.zeros((16, icols), np.int16)
        icol = 0
        for t in range(LT):
            g = t * NC + c
            # own-target gather: 128 consecutive canonical rows + 16 sentinels
            own = np.full(144, PADROW, np.int64)
            own[:128] = g * P + np.arange(P)
            idx_arr[:, icol : icol + 9] = (own - BASE).astype(
                np.int16
            ).reshape(-1, 16).T
            icol += 9

            cap = int(tile_cap[t]) + int(sent[t])
            e0, e1 = starts[g * P], starts[(g + 1) * P]
            fl = np.full(P * cap, PADROW, np.int64)
            pp = rt_s[e0:e1] - g * P
            fl[slot_of[e0:e1] * P + pp] = rs_s[e0:e1]
            if not sent[t]:
                last = (cap - 1) * P + 127
                if fl[last] < BASE:
                    own_slots = fl[127 :: P][:cap]
                    hi = np.where(own_slots >= BASE)[0]
                    assert len(hi), (t, c)
                    j = int(hi[0])
                    fl[last], fl[j * P + 127] = fl[j * P + 127], fl[last]
            i16 = (fl - BASE).astype(np.int16)
            idx_arr[:, icol : icol + 8 * cap] = i16.reshape(-1, 16).T
            icol += 8 * cap
        assert icol == icols

        xTs = np.ascontiguousarray(
            xT[:, c * SH : (c + 1) * SH]
        ).reshape(2, 128, SH)
        in_maps.append(dict(idx=idx_arr, xTs=xTs))

    # ---- weight-space folding (host, f64) — identical to v3
    W = np.asarray(W, np.float64)
    bw_ = np.asarray(bw, np.float64)
    A_ = np.asarray(A, np.float64)
    ba_ = np.asarray(ba, np.float64)
    gamma = np.asarray(gamma, np.float64)
    beta = np.asarray(beta, np.float64)
    Wout_ = np.asarray(Wout, np.float64)
    bout_ = np.asarray(bout, np.float64)

    Wcat = np.zeros((F_IN, HID))
    va1 = np.zeros((F_IN, HEADS))
    va2 = np.zeros((F_IN, HEADS))
    c1 = np.zeros(HEADS)
    c2 = np.zeros(HEADS)
    for h in range(HEADS):
        Wcat[:, h * HD : (h + 1) * HD] = W[h]
        va1[:, h] = W[h] @ A_[h, :HD]
        va2[:, h] = W[h] @ A_[h, HD:]
        c1[h] = bw_[h] @ A_[h, :HD] + ba_[h]
        c2[h] = bw_[h] @ A_[h, HD:]

    WCATA = np.zeros((2, 128, 288))
    BIASROW = np.zeros((1, 288))
    for k in range(2):
        WCATA[k, :, 0:256] = Wcat[k * 128 : (k + 1) * 128, :]
        for h in range(HEADS):
            for r in range(2):
                WCATA[k, :, 256 + 2 * h + r] = va2[k * 128 : (k + 1) * 128, h]
                WCATA[k, :, 272 + 2 * h + r] = va1[k * 128 : (k + 1) * 128, h]
    BIASROW[0, 128:256] = bw_.reshape(-1)[128:256]
    for h in range(HEADS):
        for r in range(2):
            BIASROW[0, 256 + 2 * h + r] = c2[h]
            BIASROW[0, 272 + 2 * h + r] = c1[h]

    WTILE = np.zeros((2, 128, HID))
    for f in range(F_IN):
        h, j = f // HD, f % HD
        WTILE[f // 128, f % 128, :] = gamma[h, j] * Wout_[j, :] / HEADS
    WEXTRA = np.zeros((9, HID))
    for h in range(HEADS):
        WEXTRA[h] = -(gamma[h] @ Wout_) / (HEADS * HD)
    WEXTRA[8] = bout_ + beta.mean(axis=0) @ Wout_

    padrow = np.zeros(TBW, ml_dtypes.bfloat16)
    padrow[256:272] = -240.0

    consts = dict(
        WCATA=WCATA.astype(ml_dtypes.bfloat16),
        BIASROW=BIASROW.astype(ml_dtypes.bfloat16),
        BWEXP=np.tile(bw_.reshape(1, -1), (P, 1)).astype(ml_dtypes.bfloat16),
        WTILE=WTILE.astype(ml_dtypes.bfloat16),
        WEXTRA=WEXTRA.astype(ml_dtypes.bfloat16),
        PADROW=padrow.reshape(1, TBW),
        ONESB=np.ones((1, 128), ml_dtypes.bfloat16),
        IDB=np.eye(P, dtype=ml_dtypes.bfloat16),
        IDF=np.eye(P, dtype=np.float32),
    )
    meta = dict(tile_cap=tile_cap, sent=sent, icols=icols, perm=perm)
    return meta, in_maps, consts


# ------------------------------------------------------------- device build
def _build(meta, consts):
    tile_cap, icols = meta["tile_cap"], meta["icols"]
    sent = meta["sent"]

    nc = bacc.Bacc(None, num_devices=NC)

    xTs_d = nc.dram_tensor("xTs", [2, 128, SH], BF16, kind="ExternalInput")
    idx_d = nc.dram_tensor("idx", [16, icols], I16, kind="ExternalInput")
    y_d = nc.dram_tensor("y", [LT * P, HID], BF16, kind="ExternalOutput")

    cWCATA = nc.inline_tensor(np.asarray(consts["WCATA"]), "cWCATA")
    cBIASROW = nc.inline_tensor(np.asarray(consts["BIASROW"]), "cBIASROW")
    cBWEXP = nc.inline_tensor(np.asarray(consts["BWEXP"]), "cBWEXP")
    cWTILE = nc.inline_tensor(np.asarray(consts["WTILE"]), "cWTILE")
    cWEXTRA = nc.inline_tensor(np.asarray(consts["WEXTRA"]), "cWEXTRA")
    cPADROW = nc.inline_tensor(np.asarray(consts["PADROW"]), "cPADROW")
    cONESB = nc.inline_tensor(np.asarray(consts["ONESB"]), "cONESB")
    cIDB = nc.inline_tensor(np.asarray(consts["IDB"]), "cIDB")
    cIDF = nc.inline_tensor(np.asarray(consts["IDF"]), "cIDF")

    nidx_r = nc.alloc_register(mybir.EngineType.Pool, "nidx")

    with tile.TileContext(nc) as tc:
        with (
            tc.tile_pool(name="const", bufs=1) as cpool,
            tc.tile_pool(name="dram", bufs=1, space="DRAM") as dram,
        ):
            WCATA = cpool.tile([128, 2, 288], BF16)
            BIASROW = cpool.tile([1, 288], BF16)
            BWEXP = cpool.tile([P, 256], BF16)
            WTILE = cpool.tile([128, 2, HID], BF16)
            WEXTRA = cpool.tile([9, HID], BF16)
            PADT = cpool.tile([1, TBW], BF16)
            ONESB = cpool.tile([1, 128], BF16)
            IDB = cpool.tile([P, P], BF16)
            IDF = cpool.tile([P, P], F32)
            EPSC = cpool.tile([P, 1], F32)
            IDXS = cpool.tile([32, icols], I16)
            nc.gpsimd.memset(EPSC[:], EPS)
            nc.sync.dma_start(WCATA[:], cWCATA[:].rearrange("k p n -> p k n"))
            nc.sync.dma_start(BIASROW[:], cBIASROW[:])
            nc.sync.dma_start(BWEXP[:], cBWEXP[:])
            nc.sync.dma_start(WTILE[:], cWTILE[:].rearrange("k p n -> p k n"))
            nc.sync.dma_start(WEXTRA[:], cWEXTRA[:])
            nc.sync.dma_start(PADT[:], cPADROW[:])
            nc.sync.dma_start(ONESB[:], cONESB[:])
            nc.sync.dma_start(IDB[:], cIDB[:])
            nc.sync.dma_start(IDF[:], cIDF[:])
            nc.sync.dma_start(IDXS[0:16, :], idx_d[:])
            nc.sync.dma_start(IDXS[16:32, :], idx_d[:])

            # table shard (this core's 6272 rows) + full gathered table
            tblsh = dram.tile([SH, TBW], BF16)
            tbl_d = dram.tile([NR, TBW], BF16)

            # ================= Phase A: full node table ====================
            with (
                tc.tile_pool(name="xp", bufs=4) as xpool,
                tc.tile_pool(name="ap", bufs=4) as apool,
                tc.tile_pool(name="psA", bufs=2, space="PSUM") as psA,
            ):
                batches = [(b * 512, 512) for b in range(12)] + [
                    (6144, 128)
                ]
                for off, nb in batches:
                    kb = nb // P
                    xb = xpool.tile([128, 2, 512], BF16, tag="xb")
                    nc.sync.dma_start(
                        xb[:, :, 0:nb],
                        xTs_d[:, :, off : off + nb].rearrange(
                            "k p n -> p k n"
                        ),
                    )
                    ph = psA.tile([P, 4, 512], F32, tag="phA")
                    for k in range(kb):
                        nc.tensor.matmul(
                            ph[:, k, 0:288], xb[:, 0, k * P : (k + 1) * P],
                            WCATA[:, 0, :], start=True, stop=False,
                        )
                        nc.tensor.matmul(
                            ph[:, k, 0:128], xb[:, 1, k * P : (k + 1) * P],
                            WCATA[:, 1, 0:128], start=False, stop=True,
                        )
                        nc.tensor.matmul(
                            ph[:, k, 128:256], xb[:, 1, k * P : (k + 1) * P],
                            WCATA[:, 1, 128:256], start=False, stop=False,
                        )
                        nc.tensor.matmul(
                            ph[:, k, 256:288], xb[:, 1, k * P : (k + 1) * P],
                            WCATA[:, 1, 256:288], start=False, stop=False,
                        )
                        nc.tensor.matmul(
                            ph[:, k, 128:288], ONESB[:], BIASROW[:, 128:288],
                            start=False, stop=True,
                        )
                    t8 = apool.tile([P, 4, 288], BF16, tag="t8")
                    nc.vector.tensor_tensor(
                        out=t8[:, 0:kb, 0:128],
                        in0=ph[:, 0:kb, 0:128],
                        in1=BWEXP[:, 0:128]
                        .unsqueeze(1)
                        .to_broadcast([P, kb, 128]),
                        op=OP.add,
                    )
                    nc.scalar.copy(t8[:, 0:kb, 128:288], ph[:, 0:kb, 128:288])
                    nc.sync.dma_start(
                        tblsh[off : off + nb, 0:288].rearrange(
                            "(k p) w -> p k w", p=P
                        ),
                        t8[:, 0:kb, :],
                    )
                nc.gpsimd.collective_compute(
                    "AllGather",
                    OP.bypass,
                    replica_groups=[list(range(NC))],
                    ins=[tblsh[:].opt()],
                    outs=[tbl_d[0:NPAD, :].opt()],
                )
                nc.sync.dma_start(tbl_d[PADROW : PADROW + 1, :], PADT[:])

            # ================= Phase B =====================================
            with (
                tc.tile_pool(name="sp", bufs=3) as spool,
                tc.tile_pool(name="gp", bufs=2) as gpool,
                tc.tile_pool(name="rp", bufs=2) as rpool,
                tc.tile_pool(name="pp", bufs=2) as ppool,
                tc.tile_pool(name="psB", bufs=2, space="PSUM") as psB,
                tc.tile_pool(name="psC", bufs=2, space="PSUM") as psC,
            ):
                n_pairs = (LT + 1) // 2
                # per-tile idx column offsets
                offs = []
                icol = 0
                for t in range(LT):
                    cap = int(tile_cap[t]) + int(sent[t])
                    offs.append((icol, icol + 9))
                    icol += 9 + 8 * cap
                for pi in range(n_pairs):
                    tiles = [t for t in (2 * pi, 2 * pi + 1) if t < LT]
                    ntl = len(tiles)

                    hblk = spool.tile([P, 2, 2, TBW], BF16, tag="hblk")
                    den = spool.tile([P, 2, HEADS], F32, tag="den")
                    pagg = psB.tile([P, 2, HID], F32, tag="ps_big")
                    for ti, t in enumerate(tiles):
                        cap = int(tile_cap[t]) + int(sent[t])
                        rcap = int(tile_cap[t])
                        nblk = (rcap + RB - 1) // RB
                        o0, oe = offs[t]
                        nc.gpsimd.reg_mov(nidx_r, 144)
                        nc.gpsimd.dma_gather(
                            hblk[:, ti, :, :],
                            tbl_d[BASE:, :],
                            IDXS[:, o0 : o0 + 9],
                            144,
                            nidx_r,
                            TBW,
                            single_packet=False,
                        )

                        grid = gpool.tile([P, cap, TBW], BF16, tag="grid")
                        nc.gpsimd.reg_mov(nidx_r, P * cap)
                        nc.gpsimd.dma_gather(
                            grid[:],
                            tbl_d[BASE:, :],
                            IDXS[:, oe : oe + 8 * cap],
                            P * cap,
                            nidx_r,
                            TBW,
                            single_packet=False,
                        )

                        egd = spool.tile([P, cap, 16], BF16, tag="egd")
                        nc.vector.tensor_tensor(
                            out=egd[:],
                            in0=grid[:, :, 256:272],
                            in1=hblk[:, ti, 0, 272:288]
                            .unsqueeze(1)
                            .to_broadcast([P, cap, 16]),
                            op=OP.add,
                        )
                        exd = spool.tile([P, cap, 16], BF16, tag="exd")
                        nc.scalar.activation(
                            exd[:], egd[:], AF.Prelu, alpha=SLOPE
                        )
                        nc.scalar.activation(exd[:], exd[:], AF.Exp)
                        nc.vector.tensor_reduce(
                            den[:, ti, :],
                            exd[:]
                            .rearrange("p c (h two) -> p h two c", two=2)[
                                :, :, 0, :
                            ],
                            axis=AX.X,
                            op=OP.add,
                        )

                        ci = 0
                        for bb in range(nblk):
                            j0 = bb * RB
                            nb = min(RB, rcap - j0)
                            Rc = rpool.tile([P, RB, HID], BF16, tag="R")
                            nc.vector.tensor_tensor(
                                out=Rc[:, 0:nb, :].rearrange(
                                    "p c (h f two) -> p c h f two",
                                    h=HEADS, two=2,
                                ),
                                in0=grid[:, j0 : j0 + nb, 0:256].rearrange(
                                    "p c (h f two) -> p c h f two",
                                    h=HEADS, two=2,
                                ),
                                in1=exd[:, j0 : j0 + nb, :]
                                .rearrange("p c (h two) -> p c h two", two=2)
                                .unsqueeze(3)
                                .to_broadcast([P, nb, HEADS, HD // 2, 2]),
                                op=OP.mult,
                            )
                            for j in range(nb):
                                nc.tensor.matmul(
                                    pagg[:, ti, :],
                                    IDB[:],
                                    Rc[:, j, :],
                                    start=(ci == 0),
                                    stop=(ci == rcap - 1),
                                )
                                ci += 1

                    nc.vector.tensor_scalar_max(den[:], den[:], 1e-30)
                    rden = spool.tile([P, 2, HEADS], F32, tag="rden")
                    nc.vector.reciprocal(rden[:], den[:])

                    ob = ppool.tile([P, 2, HID], BF16, tag="ob")
                    nc.vector.tensor_tensor(
                        out=ob[:, 0:ntl, :].rearrange(
                            "p t (h f) -> p t h f", h=HEADS
                        ),
                        in0=pagg[:, 0:ntl, :].rearrange(
                            "p t (h f) -> p t h f", h=HEADS
                        ),
                        in1=rden[:, 0:ntl, :]
                        .unsqueeze(3)
                        .to_broadcast([P, ntl, HEADS, HD]),
                        op=OP.mult,
                    )
                    nc.vector.tensor_tensor(
                        out=ob[:, 0:ntl, :],
                        in0=ob[:, 0:ntl, :],
                        in1=hblk[:, 0:ntl, 0, 0:256],
                        op=OP.add,
                    )
                    t1 = ppool.tile([P, 2, HID], BF16, tag="t1")
                    nc.scalar.activation(
                        t1[:, 0:ntl, :], ob[:, 0:ntl, :], AF.Relu, scale=-1.0
                    )
                    nc.scalar.activation(
                        t1[:, 0:ntl, :], t1[:, 0:ntl, :], AF.Exp, scale=-1.0
                    )
                    elu = ppool.tile([P, 2, HID], BF16, tag="elu")
                    nc.vector.scalar_tensor_tensor(
                        out=elu[:, 0:ntl, :],
                        in0=t1[:, 0:ntl, :],
                        scalar=-1.0,
                        in1=ob[:, 0:ntl, :],
                        op0=OP.add,
                        op1=OP.max,
                    )

                    nh = ntl * HEADS
                    st = ppool.tile([P, 8, 2 * HEADS], F32, tag="st")
                    r1, r2, mu2, var, sd, rr, tmp, _ = (
                        st[:, i, :] for i in range(8)
                    )
                    nc.vector.tensor_reduce(
                        r1[:, 0:nh],
                        elu[:, 0:ntl, :].rearrange(
                            "p t (h f) -> p (t h) f", f=HD
                        ),
                        axis=AX.X,
                        op=OP.add,
                    )
                    sq = ppool.tile([P, 2, HID], BF16, tag="t1")
                    nc.scalar.activation(
                        sq[:, 0:ntl, :], elu[:, 0:ntl, :], AF.Square
                    )
                    nc.vector.tensor_reduce(
                        r2[:, 0:nh],
                        sq[:, 0:ntl, :].rearrange(
                            "p t (h f) -> p (t h) f", f=HD
                        ),
                        axis=AX.X,
                        op=OP.add,
                    )
                    nc.scalar.activation(
                        mu2[:, 0:nh], r1[:, 0:nh], AF.Square, scale=1.0 / HD
                    )
                    nc.vector.scalar_tensor_tensor(
                        out=var[:, 0:nh], in0=r2[:, 0:nh], scalar=1.0 / HD,
                        in1=mu2[:, 0:nh], op0=OP.mult, op1=OP.subtract,
                    )
                    nc.scalar.activation(
                        sd[:, 0:nh], var[:, 0:nh], AF.Sqrt, bias=EPSC[:]
                    )
                    nc.vector.reciprocal(rr[:, 0:nh], sd[:, 0:nh])
                    nc.vector.tensor_tensor(
                        out=tmp[:, 0:nh], in0=r1[:, 0:nh], in1=rr[:, 0:nh],
                        op=OP.mult,
                    )

                    xw = ppool.tile([P, 2, HID], BF16, tag="xw")
                    nc.vector.tensor_tensor(
                        out=xw[:, 0:ntl, :].rearrange(
                            "p t (h f) -> p t h f", h=HEADS
                        ),
                        in0=elu[:, 0:ntl, :].rearrange(
                            "p t (h f) -> p t h f", h=HEADS
                        ),
                        in1=rr[:, 0:nh]
                        .rearrange("p (t h) -> p t h", h=HEADS)
                        .unsqueeze(3)
                        .to_broadcast([P, ntl, HEADS, HD]),
                        op=OP.mult,
                    )

                    py = psB.tile([P, 2, HID], F32, tag="ps_big")
                    yb = ppool.tile([P, 2, HID], BF16, tag="yb")
                    for ti, t in enumerate(tiles):
                        xwT = spool.tile([P, 2, P], BF16, tag="xwT")
                        for k in range(2):
                            pt = psC.tile([P, P], BF16, tag="ps_tr")
                            nc.tensor.transpose(
                                pt[:], xw[:, ti, k * P : (k + 1) * P], IDB[:]
                            )
                            nc.scalar.copy(xwT[:, k, :], pt[:])
                        t9 = spool.tile([P, 9], F32, tag="t9")
                        nc.scalar.copy(
                            t9[:, 0:8], tmp[:, ti * HEADS : (ti + 1) * HEADS]
                        )
                        nc.scalar.activation(
                            t9[:, 8:9], t9[:, 0:1], AF.Copy,
                            scale=0.0, bias=1.0,
                        )
                        ptm = psC.tile([P, P], F32, tag="ps_trf")
                        nc.tensor.transpose(ptm[0:9, :], t9[:], IDF[:])
                        exT = spool.tile([9, P], BF16, tag="exT")
                        nc.scalar.copy(exT[:], ptm[0:9, :])

                        nc.tensor.matmul(
                            py[:, ti, :], xwT[:, 0, :], WTILE[:, 0, :],
                            start=True, stop=False,
                        )
                        nc.tensor.matmul(
                            py[:, ti, :], xwT[:, 1, :], WTILE[:, 1, :],
                            start=False, stop=False,
                        )
                        nc.tensor.matmul(
                            py[:, ti, :], exT[:], WEXTRA[:],
                            start=False, stop=True,
                        )
                    nc.scalar.copy(yb[:, 0:ntl, :], py[:, 0:ntl, :])
                    t2 = ppool.tile([P, 2, HID], BF16, tag="t2")
                    nc.scalar.activation(
                        t2[:, 0:ntl, :], yb[:, 0:ntl, :], AF.Relu, scale=-1.0
                    )
                    nc.scalar.activation(
                        t2[:, 0:ntl, :], t2[:, 0:ntl, :], AF.Exp, scale=-1.0
                    )
                    ysb = ppool.tile([P, 2, HID], BF16, tag="ysb")
                    nc.vector.scalar_tensor_tensor(
                        out=ysb[:, 0:ntl, :],
                        in0=t2[:, 0:ntl, :],
                        scalar=-1.0,
                        in1=yb[:, 0:ntl, :],
                        op0=OP.add,
                        op1=OP.max,
                    )
                    for ti, t in enumerate(tiles):
                        nc.sync.dma_start(
                            y_d[t * P : (t + 1) * P, :], ysb[:, ti, :]
                        )

    nc.compile()
    return nc


# ------------------------------------------------------------------ driver
_CACHE = {}


def kernel(**inputs):
    meta, in_maps, consts = _prepare(**inputs)
    key = (
        tuple(meta["tile_cap"].tolist()),
        tuple(meta["sent"].tolist()),
    )
    if key not in _CACHE:
        _CACHE[key] = _build(meta, consts)
    nc = _CACHE[key]

    from concourse.bass_utils import run_bass_kernel_spmd

    global LAST_NC, LAST_INMAPS
    LAST_NC = nc
    LAST_INMAPS = in_maps

    res = run_bass_kernel_spmd(nc, in_maps, core_ids=list(range(NC)))
    global LAST_RESULT
    LAST_RESULT = res
    outs = res.results

    y_all = np.zeros((NPAD, HID), np.float32)
    for c in range(NC):
        g_idx = (np.arange(LT) * NC + c)[:, None] * P + np.arange(P)[None, :]
        y_all[g_idx.reshape(-1)] = outs[c]["y"].astype(np.float32)
    y = np.zeros((N_NODES, HID), np.float32)
    y[meta["perm"]] = y_all[:N_NODES]
    return y
